# revision 28
# baseline (speedup 1.0000x reference)
"""DualSlidingWindowAttention Trainium2 kernel.

Sharding: 8 cores = 2 batches x 4 head-groups. Core (b, m) owns batch b,
q-heads 8m..8m+7, kv-heads 2m, 2m+1. Host sums the 4 partial o-proj outputs
per batch (f16 partials, f32 accumulate).

Single fused pipeline per core; the Tile scheduler overlaps phases because
tiles are split at the granularity readers consume them (per token-half)
and all pools coexist in SBUF/PSUM (no reuse barriers):
  - projections with weights stationary; kT/qT land score-ready, v is
    DMA-XBAR-transposed to [kv, D] (no PE/PSUM involved).
  - block-sparse attention per (kv-group, 128-query tile): 5 kv chunks,
    scores transposed [kv, q] with the group's 4 heads in the free dim.
    Softmax: exp(s/8 - 4) on ACT (bias keeps pre-norm o in f16 range),
    mask*exp(alibi) multiplied in f16 on DVE, softmax sums via a ones
    column appended to v (free on the PE), normalization per 2-qtile batch:
    reciprocal_approx_fast + DRAM-roundtrip broadcast + in-place f16 mul.
  - o-proj in 256-token slabs as each 2-qtile batch normalizes; f16 out.

DMA discipline: few, large, contiguous-row transfers; issue split across
the two HWDGE queues (sync: xt/weights/attention shuffles; scalar:
wq/wo/mconc/v-transposes) and GPSIMD SWDGE (output writes, memsets).
All matmul operands f16 (1 cycle/row), accumulation f32 in PSUM.
"""

import sys

sys.path.insert(0, "/opt/trn_rl_repo")

import numpy as np
import concourse.bass as bass
import concourse.bacc as bacc
import concourse.mybir as mybir
import concourse.tile as tile

F32 = mybir.dt.float32
F16 = mybir.dt.float16

HID, H, HK, G, D, T = 2048, 32, 8, 4, 64, 1024
W_ATT, W_SSM = 256, 64
NQT = T // 128  # 8 query tiles
KVG = 2         # kv heads (= head groups) per core
HL = 4          # q heads per kv group
EXP_BIAS = -4.0  # exp(s/8 + EXP_BIAS): keeps pre-norm o within f16 range

# slot order: [attn_left, ssm_left, attn_full, attn_causal, ssm_causal]
SLOT_SRC = [1, 0, 1, 1, 0]       # 1 = hidden (attn window), 0 = ssm
SLOT_CHOFF = [-2, -1, -1, 0, 0]  # kv chunk offset relative to qtile
SLOT_OFF = [-256, -128, -128, 0, 0]
SLOT_WIN = [W_ATT, W_SSM, W_ATT, W_ATT, W_SSM]


def first_slot(qt):
    return {0: 3, 1: 1}.get(qt, 0)


DEBUG_DUMPS = False


def build_program():
    nc = bacc.Bacc("TRN2", target_bir_lowering=False, debug=False)

    xt_ssm = nc.declare_dram_parameter("xt_ssm", [HID, T], F16, isOutput=False)
    xt_hid = nc.declare_dram_parameter("xt_hid", [HID, T], F16, isOutput=False)
    wq = nc.declare_dram_parameter("wq", [128, 32, 512], F16, isOutput=False)
    wk = nc.declare_dram_parameter("wk", [128, 16, 128], F16, isOutput=False)
    wv = nc.declare_dram_parameter("wv", [128, 16, 128], F16, isOutput=False)
    wsk = nc.declare_dram_parameter("wsk", [128, 16, 128], F16, isOutput=False)
    wsv = nc.declare_dram_parameter("wsv", [128, 16, 128], F16, isOutput=False)
    wo = nc.declare_dram_parameter("wo", [128, 4, 2048], F16, isOutput=False)
    mconc = nc.declare_dram_parameter("mconc", [128, 10, 512], F16, isOutput=False)
    out_t = nc.declare_dram_parameter("out_t", [HID, T], F16, isOutput=True)

    mm = nc.tensor.matmul
    xt_dram = [xt_ssm, xt_hid]

    if DEBUG_DUMPS:
        dbg_kT = nc.declare_dram_parameter("dbg_kT", [128, 4, 512], F16,
                                           isOutput=True)
        dbg_qT = nc.declare_dram_parameter("dbg_qT", [128, 8, 512], F16,
                                           isOutput=True)
        dbg_v = nc.declare_dram_parameter("dbg_v", [128, 32, 65], F16,
                                          isOutput=True)
        dbg_oT = nc.declare_dram_parameter("dbg_oT", [128, 4, T], F16,
                                           isOutput=True)
        dbg_sums = nc.declare_dram_parameter("dbg_sums", [16, 512], F32,
                                             isOutput=True)
        dbg_wei = nc.declare_dram_parameter("dbg_wei", [128, 80, 512], F16,
                                            isOutput=True)

    with tile.TileContext(nc) as tc:
        with (
            tc.tile_pool(name="persist", bufs=1) as pers,
            tc.tile_pool(name="dram", bufs=1, space="DRAM") as dramp,
            tc.tile_pool(name="stp", bufs=2) as stp,
            tc.tile_pool(name="weip", bufs=2) as weip,
            tc.tile_pool(name="ostgp", bufs=2) as ostgp,
            tc.tile_pool(name="sstgp", bufs=1) as sstgp,
            tc.tile_pool(name="outstgp", bufs=2) as outstgp,
            tc.tile_pool(name="rbcp", bufs=2) as rbcp,
            tc.tile_pool(name="recp", bufs=1) as recp,
            tc.tile_pool(name="otnp", bufs=2) as otnp,
            tc.tile_pool(name="kvqp", bufs=2, space="PSUM") as kvqp,
            tc.tile_pool(name="spp", bufs=2, space="PSUM") as spp,
            tc.tile_pool(name="opp", bufs=2, space="PSUM") as opp,
        ):
            # ---- persistent tiles (each tag its own slot) ----
            # xt[src][pc]: [128, 2 chunks, 1024 tokens]; chunk kc -> (kc//2, kc%2)
            xt_sb = [
                [pers.tile([128, 2, T], F16, tag=f"xt{s}_{pc}", name=f"xt{s}_{pc}")
                 for pc in range(8)]
                for s in range(2)
            ]
            wq_sb = pers.tile([128, 32, 512], F16, tag="wq")
            w4_names = ("wsk", "wsv", "wk", "wv")
            w4_t = {"wsk": wsk, "wsv": wsv, "wk": wk, "wv": wv}
            w4_sb = {n: pers.tile([128, 16, 128], F16, tag=n, name=n)
                     for n in w4_names}
            wo_sb = pers.tile([128, 4, 2048], F16, tag="wo")
            m_sb = pers.tile([128, 10, 512], F16, tag="mconc")
            # qT[half]: [128 (kvg,d), 4 qtiles, 512 (4 heads x 128 q)]
            qT_sb = [pers.tile([128, 4, 512], F16, tag=f"qT{h}", name=f"qT{h}")
                     for h in range(2)]
            # kT[src][half]: [128 (kvg,d), 512 tokens]
            kT_sb = [[pers.tile([128, 512], F16, tag=f"kT{s}{h}", name=f"kT{s}{h}")
                      for h in range(2)] for s in range(2)]
            # v[src][kvh][half]: [128 tok-in-chunk, 4 chunks, 128]; cols 0:64
            # = v, col 64 = ones (128-wide so DMA-transpose dst offsets stay
            # 256B-aligned; cols 65:128 unused)
            v_sb = [
                [[pers.tile([128, 4, 128], F16, tag=f"v{s}{kh}{h}",
                            name=f"v{s}{kh}{h}") for h in range(2)]
                 for kh in range(2)]
                for s in range(2)
            ]
            # oT: [128 (par,d), 4 (kvg,t), 1024 tokens] f16 pre-norm;
            # normalized 256-token slabs rotate through otnp
            oT_sb = pers.tile([128, 4, T], F16, tag="oT")
            otn_tiles = {}
            # per-pair sums tiles so reciprocal reads from partition base 0
            sums_sb = [pers.tile([4, 512], F32, tag=f"sums{p}", name=f"sums{p}")
                       for p in range(4)]
            rd = dramp.tile([16, 512], F16, tag="rd")
            ebias_sb = pers.tile([128, 1], F32, tag="ebias")
            nc.gpsimd.memset(ebias_sb[:, :], EXP_BIAS)

            # ones column of v (col 64)
            for s in range(2):
                for kh in range(2):
                    for h in range(2):
                        nc.gpsimd.memset(v_sb[s][kh][h][:, :, 64:65], 1.0)

            # ---- input DMAs ----
            # kv weights first (small, gate the first matmuls), then xt
            # pieces: [128, 2, 1024] contiguous-row blocks, srcs interleaved
            # so early chunks of both sources arrive first (sync queue).
            for n in w4_names:
                nc.sync.dma_start(out=w4_sb[n], in_=w4_t[n][:, :, :])
            for pc in range(8):
                for s in range(2):
                    nc.sync.dma_start(
                        out=xt_sb[s][pc],
                        in_=xt_dram[s][pc * 256:(pc + 1) * 256, :].rearrange(
                            "(c p) t -> p c t", c=2))
            # big weight tables on the scalar HWDGE queue
            nc.scalar.dma_start(out=wq_sb, in_=wq[:, :, :])
            nc.scalar.dma_start(out=wo_sb, in_=wo[:, :, :])
            nc.scalar.dma_start(out=m_sb, in_=mconc[:, :, :])

            def xt_ap(src, kc, half):
                return xt_sb[src][kc // 2][:, kc % 2,
                                           half * 512:(half + 1) * 512]

            # ---- projections for one token half ----
            def emit_kv_proj(half, src):
                kps = kvqp.tile([128, 512], F32, tag="kv")
                vps = kvqp.tile([128, 512], F32, tag="kv")
                wk_t = w4_sb["wk" if src else "wsk"]
                wv_t = w4_sb["wv" if src else "wsv"]
                for kc in range(16):
                    xap = xt_ap(src, kc, half)
                    mm(kps[:, :], lhsT=wk_t[:, kc, :], rhs=xap,
                       start=(kc == 0), stop=(kc == 15))
                    mm(vps[:, :], lhsT=wv_t[:, kc, :], rhs=xap,
                       start=(kc == 0), stop=(kc == 15))
                nc.vector.tensor_copy(kT_sb[src][half][:, :], kps[:, :])
                stage = stp.tile([128, 512], F16, tag="stage")
                nc.vector.tensor_copy(stage[:, :], vps[:, :])
                # v: [d, tok] -> [tok, d] via DMA XBAR transpose; on sync,
                # where it queues behind the phase-A input stream and keeps
                # the scalar engine free for attention exps.
                for kh in range(2):
                    for j in range(4):
                        nc.sync.dma_start(
                            out=v_sb[src][kh][half][:, j, 0:64],
                            in_=stage[kh * 64:(kh + 1) * 64,
                                      j * 128:(j + 1) * 128],
                            transpose=True)

            def emit_q_proj(half, c):
                qps = kvqp.tile([128, 512], F32, tag="q")
                for src in range(2):
                    for kc in range(16):
                        mm(qps[:, :],
                           lhsT=wq_sb[:, src * 16 + kc, c * 128:(c + 1) * 128],
                           rhs=xt_ap(src, kc, half),
                           start=(src == 0 and kc == 0),
                           stop=(src == 1 and kc == 15))
                nc.vector.tensor_copy(
                    qT_sb[half][:, :, c * 128:(c + 1) * 128],
                    qps[:, :].rearrange("p (qt j) -> p qt j", j=128))

            # ---- attention ----
            units = [(qt * 2 + kvg, kvg, qt) for qt in range(NQT)
                     for kvg in range(KVG)]
            wei_tiles = {}

            def emit_scores(u):
                _, kvg, qt = units[u]
                fs = first_slot(qt)
                wei_t = weip.tile([128, 5, 512], F16, tag="wei")
                wei_tiles[u] = wei_t
                for s in range(fs, 5):
                    ch = qt + SLOT_CHOFF[s]
                    src = SLOT_SRC[s]
                    sp_t = spp.tile([128, 512], F32, tag="sp")
                    mm(sp_t[:, :],
                       lhsT=kT_sb[src][ch // 4][kvg * 64:(kvg + 1) * 64,
                                                (ch % 4) * 128:(ch % 4 + 1) * 128],
                       rhs=qT_sb[qt // 4][kvg * 64:(kvg + 1) * 64, qt % 4, :],
                       start=True, stop=True)
                    nc.scalar.activation(
                        out=wei_t[:, s, :], in_=sp_t[:, :],
                        func=mybir.ActivationFunctionType.Exp,
                        scale=0.125, bias=ebias_sb[:, :])
                nc.vector.tensor_mul(
                    wei_t[:, fs:5, :], wei_t[:, fs:5, :],
                    m_sb[:, kvg * 5 + fs:kvg * 5 + 5, :])
                if DEBUG_DUMPS:
                    nc.gpsimd.dma_start(
                        out=dbg_wei[:, u * 5 + fs:u * 5 + 5, :],
                        in_=wei_t[:, fs:5, :])

            def emit_o(u):
                _, kvg, qt = units[u]
                fs = first_slot(qt)
                wei_t = wei_tiles.pop(u)
                op_t = opp.tile([128, 512], F32, tag="op")
                for s in range(fs, 5):
                    ch = qt + SLOT_CHOFF[s]
                    src = SLOT_SRC[s]
                    mm(op_t[0:65, :],
                       lhsT=v_sb[src][kvg][ch // 4][:, ch % 4, 0:65],
                       rhs=wei_t[:, s, :],
                       start=(s == fs), stop=(s == 4))
                ostg = ostgp.tile([64, 512], F16, tag="ostg")
                nc.vector.tensor_copy(ostg[:, :], op_t[0:64, :])
                # sums row: psum f32 -> f32 staging at same partition base,
                # then DMA into this pair's sums tile
                sstg = sstgp.tile([65, 512], F32, tag="sstg")
                nc.vector.tensor_copy(sstg[64:65, :], op_t[64:65, :])
                nc.sync.dma_start(out=sums_sb[u // 4][u % 4:u % 4 + 1, :],
                                  in_=sstg[64:65, :])
                for par in range(2):
                    src_ap = ostg[:, :].rearrange(
                        "p (t pr j) -> p t pr j", t=2, pr=2)[:, :, par, :]
                    dst_ap = oT_sb[par * 64:(par + 1) * 64,
                                   kvg * 2:kvg * 2 + 2,
                                   qt * 128:(qt + 1) * 128]
                    nc.sync.dma_start(out=dst_ap, in_=src_ap)

            def emit_norm(p):
                # qtiles 2p, 2p+1 = units 4p..4p+3 (sums rows likewise)
                r32 = recp.tile([4, 512], F32, tag="rf32")
                r16 = recp.tile([4, 512], F16, tag="rf16")
                nc.vector.reciprocal_approx_fast(
                    out=r32[:, :], in_=sums_sb[p][:, :])
                nc.vector.tensor_copy(r16[:, :], r32[:, :])
                nc.sync.dma_start(out=rd[4 * p:4 * p + 4, :], in_=r16[:, :])
                rbc_t = rbcp.tile([128, 4, 256], F16, tag="rbc")
                rd_ap = rd[:, :]
                for par in range(2):
                    for qtloc in range(2):
                        # value for (d, c=(kvg,t), j) at rd row
                        # 4p + 2*qtloc + kvg, col t*256 + par*128 + j;
                        # c strides uniformly by 256 (kvg row-stride 512 =
                        # 2x the t col-stride 256), so 3 dims suffice.
                        in_ap = bass.AP(
                            tensor=rd_ap.tensor,
                            offset=(rd_ap.offset + (4 * p + 2 * qtloc) * 512
                                    + par * 128),
                            ap=[[0, 64], [256, 4], [1, 128]])
                        nc.sync.dma_start(
                            out=rbc_t[par * 64:(par + 1) * 64, :,
                                      qtloc * 128:(qtloc + 1) * 128],
                            in_=in_ap)
                otn = otnp.tile([128, 4, 256], F16, tag="otn")
                otn_tiles[p] = otn
                nc.vector.tensor_mul(
                    otn[:, :, :],
                    oT_sb[:, :, p * 256:(p + 1) * 256],
                    rbc_t[:, :, :])

            def emit_oproj(p):
                # tokens [256p, 256p+256): 16 n-chunks of 128 output cols
                otn = otn_tiles.pop(p)
                for g in range(4):
                    outstg = outstgp.tile([128, 4, 256], F16, tag="outstg")
                    for i in range(4):
                        n = g * 4 + i
                        p3 = kvqp.tile([128, 256], F32, tag="kv")
                        for c in range(4):
                            mm(p3[:, :],
                               lhsT=wo_sb[:, c, n * 128:(n + 1) * 128],
                               rhs=otn[:, c, :],
                               start=(c == 0), stop=(c == 3))
                        if i % 2 == 0:
                            nc.scalar.copy(outstg[:, i, :], p3[:, :])
                        else:
                            nc.vector.tensor_copy(outstg[:, i, :], p3[:, :])
                    dst = out_t[g * 512:(g + 1) * 512,
                                p * 256:(p + 1) * 256].rearrange(
                                    "(i p2) t -> p2 i t", i=4)
                    nc.gpsimd.dma_start(out=dst, in_=outstg[:, :, :])

            # ---------------- emission schedule ----------------
            # half 0 projections
            for src in range(2):
                emit_kv_proj(0, src)
            for c in range(4):
                emit_q_proj(0, c)

            # half-1 projections interleaved with attention on qtiles 0-3;
            # the scheduler refines this, the order sets priorities.
            emit_kv_proj(1, 0)
            emit_scores(0)
            emit_kv_proj(1, 1)
            emit_scores(1)
            emit_o(0)
            emit_q_proj(1, 0)
            emit_scores(2)
            emit_o(1)
            emit_q_proj(1, 1)
            emit_scores(3)
            emit_o(2)
            emit_q_proj(1, 2)
            emit_scores(4)
            emit_o(3)
            emit_norm(0)
            emit_q_proj(1, 3)
            emit_scores(5)
            emit_o(4)
            emit_scores(6)
            emit_o(5)
            emit_scores(7)
            emit_o(6)

            # qtiles 4-7 + norms + o-proj slabs pipelined
            emit_scores(8)
            emit_o(7)
            emit_norm(1)
            emit_oproj(0)
            for u in range(9, 16):
                emit_scores(u)
                emit_o(u - 1)
                if u == 11:
                    emit_oproj(1)
                if u == 13:
                    emit_norm(2)
                if u == 14:
                    emit_oproj(2)
            emit_o(15)
            emit_norm(3)
            emit_oproj(3)

            if DEBUG_DUMPS:
                for s in range(2):
                    for h in range(2):
                        nc.gpsimd.dma_start(
                            out=dbg_kT[:, s * 2 + h, :], in_=kT_sb[s][h][:, :])
                for h in range(2):
                    nc.gpsimd.dma_start(
                        out=dbg_qT[:, h * 4:(h + 1) * 4, :],
                        in_=qT_sb[h][:, :, :])
                for s in range(2):
                    for kh in range(2):
                        for h in range(2):
                            idx = (s * 2 + kh) * 2 + h
                            nc.gpsimd.dma_start(
                                out=dbg_v[:, idx * 4:(idx + 1) * 4, :],
                                in_=v_sb[s][kh][h][:, :, 0:65])
                nc.gpsimd.dma_start(out=dbg_oT[:, :, :], in_=oT_sb[:, :, :])
                for p in range(4):
                    nc.gpsimd.dma_start(out=dbg_sums[4 * p:4 * p + 4, :],
                                        in_=sums_sb[p][:, :])

    nc.finalize()
    return nc


def make_mconc(m):
    """Mask*exp(alibi) tile for core head-group m: [128, 10, 512] f16."""
    p = np.arange(128)[:, None]
    j = np.arange(128)[None, :]
    out = np.zeros((128, 10, 512), np.float16)
    for kvg in range(KVG):
        for s in range(5):
            rel = SLOT_OFF[s] + p - j  # [128, 128] kv - q
            mask = (-rel >= 0) & (-rel < SLOT_WIN[s])
            for hl in range(HL):
                hg = 8 * m + kvg * 4 + hl
                slope = 2.0 ** (-8.0 * hg / H)
                vals = np.where(mask, np.exp(slope * rel.astype(np.float64)), 0.0)
                out[:, kvg * 5 + s, hl * 128:(hl + 1) * 128] = vals.astype(np.float16)
    return out


def make_inputs(core, hidden_states, ssm_states, Wq, Wk, Wv, Wsk, Wsv, Wo):
    b, m = core // 4, core % 4
    f16 = lambda x: np.ascontiguousarray(np.asarray(x, dtype=np.float16))

    def wshard(W, cols, nchunk):
        # [K, cols] -> [128, K//128, cols]
        Ws = np.asarray(W)[:, cols]
        return f16(Ws.reshape(nchunk, 128, Ws.shape[1]).transpose(1, 0, 2))

    # col-tile c = [head c (kvg0) cols, head 4+c (kvg1) cols]
    qperm = np.concatenate(
        [np.arange(64) + 64 * h for c in range(4) for h in (c, 4 + c)])
    qcols = 512 * m + qperm
    kvcols = slice(128 * m, 128 * (m + 1))
    wo_sh = np.asarray(Wo)[512 * m:512 * (m + 1), :]
    return {
        "xt_ssm": f16(np.asarray(ssm_states)[b].T),
        "xt_hid": f16(np.asarray(hidden_states)[b].T),
        "wq": wshard(Wq, qcols, 32),
        "wk": wshard(Wk, kvcols, 16),
        "wv": wshard(Wv, kvcols, 16),
        "wsk": wshard(Wsk, kvcols, 16),
        "wsv": wshard(Wsv, kvcols, 16),
        "wo": f16(wo_sh.reshape(4, 128, 2048).transpose(1, 0, 2)),
        "mconc": make_mconc(m),
    }


def gather(results):
    out = np.zeros((2, T, HID), np.float32)
    for core in range(8):
        b = core // 4
        out[b] += results[core]["out_t"].astype(np.float32).T
    return out


# ----------------------------------------------------------------------------
# Harness entry point
# ----------------------------------------------------------------------------
_NC_CACHE = []


def _get_program():
    if not _NC_CACHE:
        _NC_CACHE.append(build_program())
    return _NC_CACHE[0]


def _run(inp, trace=False):
    from concourse.bass_utils import run_bass_kernel_spmd

    nc = _get_program()
    in_maps = [make_inputs(core, **{k: np.asarray(inp[k]) for k in (
        "hidden_states", "ssm_states", "Wq", "Wk", "Wv", "Wsk", "Wsv", "Wo")})
        for core in range(8)]
    res = run_bass_kernel_spmd(nc, in_maps, list(range(8)), trace=trace)
    return gather(res.results), res.exec_time_ns


def kernel(hidden_states, ssm_states, Wq, Wk, Wv, Wsk, Wsv, Wo):
    out, _ = _run(dict(
        hidden_states=hidden_states, ssm_states=ssm_states, Wq=Wq, Wk=Wk,
        Wv=Wv, Wsk=Wsk, Wsv=Wsv, Wo=Wo))
    return out


# revision 31
# speedup vs baseline: 1.0483x; 1.0483x over previous
"""DualSlidingWindowAttention Trainium2 kernel.

Sharding: 8 cores = 2 batches x 4 head-groups. Core (b, m) owns batch b,
q-heads 8m..8m+7, kv-heads 2m, 2m+1. Host sums the 4 partial o-proj outputs
per batch (f16 partials, f32 accumulate).

Single fused pipeline per core; the Tile scheduler overlaps phases because
tiles are split at the granularity readers consume them (per token-half)
and all pools coexist in SBUF/PSUM (no reuse barriers):
  - projections with weights stationary; kT/qT land score-ready, v is
    DMA-XBAR-transposed to [kv, D] (no PE/PSUM involved).
  - block-sparse attention per (kv-group, 128-query tile): 5 kv chunks,
    scores transposed [kv, q] with the group's 4 heads in the free dim.
    Softmax: exp(s/8 - 4) on ACT (bias keeps pre-norm o in f16 range),
    mask*exp(alibi) multiplied in f16 on DVE, softmax sums via a ones
    column appended to v (free on the PE), normalization per 2-qtile batch:
    reciprocal_approx_fast + DRAM-roundtrip broadcast + in-place f16 mul.
  - o-proj in 256-token slabs as each 2-qtile batch normalizes; f16 out.

DMA discipline: few, large, contiguous-row transfers; issue split across
the two HWDGE queues (sync: xt/weights/attention shuffles; scalar:
wq/wo/mconc/v-transposes) and GPSIMD SWDGE (output writes, memsets).
All matmul operands f16 (1 cycle/row), accumulation f32 in PSUM.
"""

import sys

sys.path.insert(0, "/opt/trn_rl_repo")

import numpy as np
import concourse.bass as bass
import concourse.bacc as bacc
import concourse.mybir as mybir
import concourse.tile as tile

F32 = mybir.dt.float32
F16 = mybir.dt.float16

HID, H, HK, G, D, T = 2048, 32, 8, 4, 64, 1024
W_ATT, W_SSM = 256, 64
NQT = T // 128  # 8 query tiles
KVG = 2         # kv heads (= head groups) per core
HL = 4          # q heads per kv group
EXP_BIAS = -4.0  # exp(s/8 + EXP_BIAS): keeps pre-norm o within f16 range

# slot order: [attn_left, ssm_left, attn_full, attn_causal, ssm_causal]
SLOT_SRC = [1, 0, 1, 1, 0]       # 1 = hidden (attn window), 0 = ssm
SLOT_CHOFF = [-2, -1, -1, 0, 0]  # kv chunk offset relative to qtile
SLOT_OFF = [-256, -128, -128, 0, 0]
SLOT_WIN = [W_ATT, W_SSM, W_ATT, W_ATT, W_SSM]


def first_slot(qt):
    return {0: 3, 1: 1}.get(qt, 0)


DEBUG_DUMPS = False


def build_program():
    nc = bacc.Bacc("TRN2", target_bir_lowering=False, debug=False)

    xt_ssm = nc.declare_dram_parameter("xt_ssm", [HID, T], F16, isOutput=False)
    xt_hid = nc.declare_dram_parameter("xt_hid", [HID, T], F16, isOutput=False)
    wq = nc.declare_dram_parameter("wq", [128, 32, 512], F16, isOutput=False)
    wk = nc.declare_dram_parameter("wk", [128, 16, 128], F16, isOutput=False)
    wv = nc.declare_dram_parameter("wv", [128, 16, 128], F16, isOutput=False)
    wsk = nc.declare_dram_parameter("wsk", [128, 16, 128], F16, isOutput=False)
    wsv = nc.declare_dram_parameter("wsv", [128, 16, 128], F16, isOutput=False)
    wo = nc.declare_dram_parameter("wo", [128, 4, 2048], F16, isOutput=False)
    mconc = nc.declare_dram_parameter("mconc", [128, 10, 512], F16, isOutput=False)
    out_t = nc.declare_dram_parameter("out_t", [HID, T], F16, isOutput=True)

    mm = nc.tensor.matmul
    xt_dram = [xt_ssm, xt_hid]

    if DEBUG_DUMPS:
        dbg_kT = nc.declare_dram_parameter("dbg_kT", [128, 4, 512], F16,
                                           isOutput=True)
        dbg_qT = nc.declare_dram_parameter("dbg_qT", [128, 8, 512], F16,
                                           isOutput=True)
        dbg_v = nc.declare_dram_parameter("dbg_v", [128, 32, 65], F16,
                                          isOutput=True)
        dbg_oT = nc.declare_dram_parameter("dbg_oT", [128, 4, T], F16,
                                           isOutput=True)
        dbg_sums = nc.declare_dram_parameter("dbg_sums", [16, 512], F32,
                                             isOutput=True)
        dbg_wei = nc.declare_dram_parameter("dbg_wei", [128, 80, 512], F16,
                                            isOutput=True)

    with tile.TileContext(nc) as tc:
        with (
            tc.tile_pool(name="persist", bufs=1) as pers,
            tc.tile_pool(name="dram", bufs=1, space="DRAM") as dramp,
            tc.tile_pool(name="stp", bufs=2) as stp,
            tc.tile_pool(name="weip", bufs=2) as weip,
            tc.tile_pool(name="ostgp", bufs=2) as ostgp,
            tc.tile_pool(name="sstgp", bufs=1) as sstgp,
            tc.tile_pool(name="outstgp", bufs=2) as outstgp,
            tc.tile_pool(name="rbcp", bufs=2) as rbcp,
            tc.tile_pool(name="recp", bufs=1) as recp,
            tc.tile_pool(name="otnp", bufs=2) as otnp,
            tc.tile_pool(name="kvqp", bufs=2, space="PSUM") as kvqp,
            tc.tile_pool(name="spp", bufs=2, space="PSUM") as spp,
            tc.tile_pool(name="opp", bufs=2, space="PSUM") as opp,
        ):
            # ---- persistent tiles (each tag its own slot) ----
            # xt[src][pc]: [128, 2 chunks, 1024 tokens]; chunk kc -> (kc//2, kc%2)
            xt_sb = [
                [pers.tile([128, 2, T], F16, tag=f"xt{s}_{pc}", name=f"xt{s}_{pc}")
                 for pc in range(8)]
                for s in range(2)
            ]
            wq_sb = pers.tile([128, 32, 512], F16, tag="wq")
            w4_names = ("wsk", "wsv", "wk", "wv")
            w4_t = {"wsk": wsk, "wsv": wsv, "wk": wk, "wv": wv}
            w4_sb = {n: pers.tile([128, 16, 128], F16, tag=n, name=n)
                     for n in w4_names}
            wo_sb = pers.tile([128, 4, 2048], F16, tag="wo")
            m_sb = pers.tile([128, 10, 512], F16, tag="mconc")
            # qT[half]: [128 (kvg,d), 4 qtiles, 512 (4 heads x 128 q)]
            qT_sb = [pers.tile([128, 4, 512], F16, tag=f"qT{h}", name=f"qT{h}")
                     for h in range(2)]
            # kT[src][half]: [128 (kvg,d), 512 tokens]
            kT_sb = [[pers.tile([128, 512], F16, tag=f"kT{s}{h}", name=f"kT{s}{h}")
                      for h in range(2)] for s in range(2)]
            # v[src][kvh][half]: [128 tok-in-chunk, 4 chunks, 128]; cols 0:64
            # = v, col 64 = ones (128-wide so DMA-transpose dst offsets stay
            # 256B-aligned; cols 65:128 unused)
            v_sb = [
                [[pers.tile([128, 4, 128], F16, tag=f"v{s}{kh}{h}",
                            name=f"v{s}{kh}{h}") for h in range(2)]
                 for kh in range(2)]
                for s in range(2)
            ]
            # oT: [128 (par,d), 4 (kvg,t), 1024 tokens] f16 pre-norm;
            # normalized 256-token slabs rotate through otnp
            oT_sb = pers.tile([128, 4, T], F16, tag="oT")
            otn_tiles = {}
            # per-pair sums tiles so reciprocal reads from partition base 0
            sums_sb = [pers.tile([4, 512], F32, tag=f"sums{p}", name=f"sums{p}")
                       for p in range(4)]
            rd = dramp.tile([16, 512], F16, tag="rd")
            ebias_sb = pers.tile([128, 1], F32, tag="ebias")
            nc.gpsimd.memset(ebias_sb[:, :], EXP_BIAS)

            # ones column of v (col 64)
            for s in range(2):
                for kh in range(2):
                    for h in range(2):
                        nc.gpsimd.memset(v_sb[s][kh][h][:, :, 64:65], 1.0)

            # ---- input DMAs ----
            # kv weights first (small, gate the first matmuls), then xt
            # pieces: [128, 2, 1024] contiguous-row blocks, srcs interleaved
            # so early chunks of both sources arrive first (sync queue).
            for n in w4_names:
                nc.sync.dma_start(out=w4_sb[n], in_=w4_t[n][:, :, :])
            for pc in range(8):
                for s in range(2):
                    nc.sync.dma_start(
                        out=xt_sb[s][pc],
                        in_=xt_dram[s][pc * 256:(pc + 1) * 256, :].rearrange(
                            "(c p) t -> p c t", c=2))
            # big weight tables on the scalar HWDGE queue; wq split in 4 so
            # q-proj matmuls unlock as chunks land, wo (needed last) last
            for w in range(4):
                nc.scalar.dma_start(out=wq_sb[:, w * 8:(w + 1) * 8, :],
                                    in_=wq[:, w * 8:(w + 1) * 8, :])
            nc.scalar.dma_start(out=m_sb, in_=mconc[:, :, :])
            nc.scalar.dma_start(out=wo_sb, in_=wo[:, :, :])

            def xt_ap(src, kc, half):
                return xt_sb[src][kc // 2][:, kc % 2,
                                           half * 512:(half + 1) * 512]

            # ---- projections for one token half ----
            def emit_kv_proj(half, src):
                kps = kvqp.tile([128, 512], F32, tag="kv")
                vps = kvqp.tile([128, 512], F32, tag="kv")
                wk_t = w4_sb["wk" if src else "wsk"]
                wv_t = w4_sb["wv" if src else "wsv"]
                for kc in range(16):
                    xap = xt_ap(src, kc, half)
                    mm(kps[:, :], lhsT=wk_t[:, kc, :], rhs=xap,
                       start=(kc == 0), stop=(kc == 15))
                    mm(vps[:, :], lhsT=wv_t[:, kc, :], rhs=xap,
                       start=(kc == 0), stop=(kc == 15))
                nc.vector.tensor_copy(kT_sb[src][half][:, :], kps[:, :])
                stage = stp.tile([128, 512], F16, tag="stage")
                nc.vector.tensor_copy(stage[:, :], vps[:, :])
                # v: [d, tok] -> [tok, d] via DMA XBAR transpose; on sync,
                # where it queues behind the phase-A input stream and keeps
                # the scalar engine free for attention exps.
                for kh in range(2):
                    for j in range(4):
                        nc.sync.dma_start(
                            out=v_sb[src][kh][half][:, j, 0:64],
                            in_=stage[kh * 64:(kh + 1) * 64,
                                      j * 128:(j + 1) * 128],
                            transpose=True)

            def emit_q_proj(half, c):
                qps = kvqp.tile([128, 512], F32, tag="q")
                for src in range(2):
                    for kc in range(16):
                        mm(qps[:, :],
                           lhsT=wq_sb[:, src * 16 + kc, c * 128:(c + 1) * 128],
                           rhs=xt_ap(src, kc, half),
                           start=(src == 0 and kc == 0),
                           stop=(src == 1 and kc == 15))
                nc.vector.tensor_copy(
                    qT_sb[half][:, :, c * 128:(c + 1) * 128],
                    qps[:, :].rearrange("p (qt j) -> p qt j", j=128))

            # ---- attention ----
            units = [(qt * 2 + kvg, kvg, qt) for qt in range(NQT)
                     for kvg in range(KVG)]
            wei_tiles = {}

            def emit_scores(u):
                _, kvg, qt = units[u]
                fs = first_slot(qt)
                wei_t = weip.tile([128, 5, 512], F16, tag="wei")
                wei_tiles[u] = wei_t
                for s in range(fs, 5):
                    ch = qt + SLOT_CHOFF[s]
                    src = SLOT_SRC[s]
                    sp_t = spp.tile([128, 512], F32, tag="sp")
                    mm(sp_t[:, :],
                       lhsT=kT_sb[src][ch // 4][kvg * 64:(kvg + 1) * 64,
                                                (ch % 4) * 128:(ch % 4 + 1) * 128],
                       rhs=qT_sb[qt // 4][kvg * 64:(kvg + 1) * 64, qt % 4, :],
                       start=True, stop=True)
                    nc.scalar.activation(
                        out=wei_t[:, s, :], in_=sp_t[:, :],
                        func=mybir.ActivationFunctionType.Exp,
                        scale=0.125, bias=ebias_sb[:, :])
                nc.vector.tensor_mul(
                    wei_t[:, fs:5, :], wei_t[:, fs:5, :],
                    m_sb[:, kvg * 5 + fs:kvg * 5 + 5, :])
                if DEBUG_DUMPS:
                    nc.gpsimd.dma_start(
                        out=dbg_wei[:, u * 5 + fs:u * 5 + 5, :],
                        in_=wei_t[:, fs:5, :])

            def emit_o(u):
                _, kvg, qt = units[u]
                fs = first_slot(qt)
                wei_t = wei_tiles.pop(u)
                op_t = opp.tile([128, 512], F32, tag="op")
                for s in range(fs, 5):
                    ch = qt + SLOT_CHOFF[s]
                    src = SLOT_SRC[s]
                    mm(op_t[0:65, :],
                       lhsT=v_sb[src][kvg][ch // 4][:, ch % 4, 0:65],
                       rhs=wei_t[:, s, :],
                       start=(s == fs), stop=(s == 4))
                ostg = ostgp.tile([64, 512], F16, tag="ostg")
                nc.vector.tensor_copy(ostg[:, :], op_t[0:64, :])
                # sums row: psum f32 -> f32 staging at same partition base,
                # then DMA into this pair's sums tile
                sstg = sstgp.tile([65, 512], F32, tag="sstg")
                nc.vector.tensor_copy(sstg[64:65, :], op_t[64:65, :])
                nc.sync.dma_start(out=sums_sb[u // 4][u % 4:u % 4 + 1, :],
                                  in_=sstg[64:65, :])
                for par in range(2):
                    src_ap = ostg[:, :].rearrange(
                        "p (t pr j) -> p t pr j", t=2, pr=2)[:, :, par, :]
                    dst_ap = oT_sb[par * 64:(par + 1) * 64,
                                   kvg * 2:kvg * 2 + 2,
                                   qt * 128:(qt + 1) * 128]
                    nc.sync.dma_start(out=dst_ap, in_=src_ap)

            def emit_norm(p):
                # qtiles 2p, 2p+1 = units 4p..4p+3 (sums rows likewise)
                r32 = recp.tile([4, 512], F32, tag="rf32")
                r16 = recp.tile([4, 512], F16, tag="rf16")
                nc.vector.reciprocal_approx_fast(
                    out=r32[:, :], in_=sums_sb[p][:, :])
                nc.vector.tensor_copy(r16[:, :], r32[:, :])
                nc.sync.dma_start(out=rd[4 * p:4 * p + 4, :], in_=r16[:, :])
                rbc_t = rbcp.tile([128, 4, 256], F16, tag="rbc")
                rd_ap = rd[:, :]
                for par in range(2):
                    for qtloc in range(2):
                        # value for (d, c=(kvg,t), j) at rd row
                        # 4p + 2*qtloc + kvg, col t*256 + par*128 + j;
                        # c strides uniformly by 256 (kvg row-stride 512 =
                        # 2x the t col-stride 256), so 3 dims suffice.
                        in_ap = bass.AP(
                            tensor=rd_ap.tensor,
                            offset=(rd_ap.offset + (4 * p + 2 * qtloc) * 512
                                    + par * 128),
                            ap=[[0, 64], [256, 4], [1, 128]])
                        nc.sync.dma_start(
                            out=rbc_t[par * 64:(par + 1) * 64, :,
                                      qtloc * 128:(qtloc + 1) * 128],
                            in_=in_ap)
                otn = otnp.tile([128, 4, 256], F16, tag="otn")
                otn_tiles[p] = otn
                nc.vector.tensor_mul(
                    otn[:, :, :],
                    oT_sb[:, :, p * 256:(p + 1) * 256],
                    rbc_t[:, :, :])

            def emit_oproj_g(p, g):
                # tokens [256p, 256p+256), output cols [512g, 512g+512)
                otn = otn_tiles[p]
                outstg = outstgp.tile([128, 4, 256], F16, tag="outstg")
                for i in range(4):
                    n = g * 4 + i
                    p3 = kvqp.tile([128, 256], F32, tag="kv")
                    for c in range(4):
                        mm(p3[:, :],
                           lhsT=wo_sb[:, c, n * 128:(n + 1) * 128],
                           rhs=otn[:, c, :],
                           start=(c == 0), stop=(c == 3))
                    if i % 2 == 0:
                        nc.scalar.copy(outstg[:, i, :], p3[:, :])
                    else:
                        nc.vector.tensor_copy(outstg[:, i, :], p3[:, :])
                dst = out_t[g * 512:(g + 1) * 512,
                            p * 256:(p + 1) * 256].rearrange(
                                "(i p2) t -> p2 i t", i=4)
                nc.gpsimd.dma_start(out=dst, in_=outstg[:, :, :])

            def emit_oproj(p):
                for g in range(4):
                    emit_oproj_g(p, g)
                del otn_tiles[p]

            # ---------------- emission schedule ----------------
            # half 0 projections
            for src in range(2):
                emit_kv_proj(0, src)
            for c in range(4):
                emit_q_proj(0, c)

            # half-1 projections interleaved with attention on qtiles 0-3;
            # the scheduler refines this, the order sets priorities.
            emit_kv_proj(1, 0)
            emit_scores(0)
            emit_kv_proj(1, 1)
            emit_scores(1)
            emit_o(0)
            emit_q_proj(1, 0)
            emit_scores(2)
            emit_o(1)
            emit_q_proj(1, 1)
            emit_scores(3)
            emit_o(2)
            emit_q_proj(1, 2)
            emit_scores(4)
            emit_o(3)
            emit_norm(0)
            emit_q_proj(1, 3)
            emit_scores(5)
            emit_o(4)
            emit_scores(6)
            emit_o(5)
            emit_scores(7)
            emit_o(6)

            # qtiles 4-7 + norms + o-proj slabs pipelined; o-proj emitted in
            # 16-matmul groups between units to keep every engine fed
            emit_scores(8)
            emit_o(7)
            emit_norm(1)
            emit_oproj_g(0, 0)
            emit_oproj_g(0, 1)
            sched = {9: [(0, 2), (0, 3)], 10: [(1, 0), (1, 1)],
                     11: [(1, 2)], 12: [(1, 3)],
                     13: [(2, 0)], 14: [(2, 1), (2, 2)], 15: [(2, 3)]}
            for u in range(9, 16):
                emit_scores(u)
                emit_o(u - 1)
                if u == 13:
                    emit_norm(2)
                for (pp, g) in sched.get(u, []):
                    emit_oproj_g(pp, g)
            emit_o(15)
            emit_norm(3)
            emit_oproj(3)

            if DEBUG_DUMPS:
                for s in range(2):
                    for h in range(2):
                        nc.gpsimd.dma_start(
                            out=dbg_kT[:, s * 2 + h, :], in_=kT_sb[s][h][:, :])
                for h in range(2):
                    nc.gpsimd.dma_start(
                        out=dbg_qT[:, h * 4:(h + 1) * 4, :],
                        in_=qT_sb[h][:, :, :])
                for s in range(2):
                    for kh in range(2):
                        for h in range(2):
                            idx = (s * 2 + kh) * 2 + h
                            nc.gpsimd.dma_start(
                                out=dbg_v[:, idx * 4:(idx + 1) * 4, :],
                                in_=v_sb[s][kh][h][:, :, 0:65])
                nc.gpsimd.dma_start(out=dbg_oT[:, :, :], in_=oT_sb[:, :, :])
                for p in range(4):
                    nc.gpsimd.dma_start(out=dbg_sums[4 * p:4 * p + 4, :],
                                        in_=sums_sb[p][:, :])

    nc.finalize()
    return nc


def make_mconc(m):
    """Mask*exp(alibi) tile for core head-group m: [128, 10, 512] f16."""
    p = np.arange(128)[:, None]
    j = np.arange(128)[None, :]
    out = np.zeros((128, 10, 512), np.float16)
    for kvg in range(KVG):
        for s in range(5):
            rel = SLOT_OFF[s] + p - j  # [128, 128] kv - q
            mask = (-rel >= 0) & (-rel < SLOT_WIN[s])
            for hl in range(HL):
                hg = 8 * m + kvg * 4 + hl
                slope = 2.0 ** (-8.0 * hg / H)
                vals = np.where(mask, np.exp(slope * rel.astype(np.float64)), 0.0)
                out[:, kvg * 5 + s, hl * 128:(hl + 1) * 128] = vals.astype(np.float16)
    return out


def make_inputs(core, hidden_states, ssm_states, Wq, Wk, Wv, Wsk, Wsv, Wo):
    b, m = core // 4, core % 4
    f16 = lambda x: np.ascontiguousarray(np.asarray(x, dtype=np.float16))

    def wshard(W, cols, nchunk):
        # [K, cols] -> [128, K//128, cols]
        Ws = np.asarray(W)[:, cols]
        return f16(Ws.reshape(nchunk, 128, Ws.shape[1]).transpose(1, 0, 2))

    # col-tile c = [head c (kvg0) cols, head 4+c (kvg1) cols]
    qperm = np.concatenate(
        [np.arange(64) + 64 * h for c in range(4) for h in (c, 4 + c)])
    qcols = 512 * m + qperm
    kvcols = slice(128 * m, 128 * (m + 1))
    wo_sh = np.asarray(Wo)[512 * m:512 * (m + 1), :]
    return {
        "xt_ssm": f16(np.asarray(ssm_states)[b].T),
        "xt_hid": f16(np.asarray(hidden_states)[b].T),
        "wq": wshard(Wq, qcols, 32),
        "wk": wshard(Wk, kvcols, 16),
        "wv": wshard(Wv, kvcols, 16),
        "wsk": wshard(Wsk, kvcols, 16),
        "wsv": wshard(Wsv, kvcols, 16),
        "wo": f16(wo_sh.reshape(4, 128, 2048).transpose(1, 0, 2)),
        "mconc": make_mconc(m),
    }


def gather(results):
    out = np.zeros((2, T, HID), np.float32)
    for core in range(8):
        b = core // 4
        out[b] += results[core]["out_t"].astype(np.float32).T
    return out


# ----------------------------------------------------------------------------
# Harness entry point
# ----------------------------------------------------------------------------
_NC_CACHE = []


def _get_program():
    if not _NC_CACHE:
        _NC_CACHE.append(build_program())
    return _NC_CACHE[0]


def _run(inp, trace=False):
    from concourse.bass_utils import run_bass_kernel_spmd

    nc = _get_program()
    in_maps = [make_inputs(core, **{k: np.asarray(inp[k]) for k in (
        "hidden_states", "ssm_states", "Wq", "Wk", "Wv", "Wsk", "Wsv", "Wo")})
        for core in range(8)]
    res = run_bass_kernel_spmd(nc, in_maps, list(range(8)), trace=trace)
    return gather(res.results), res.exec_time_ns


def kernel(hidden_states, ssm_states, Wq, Wk, Wv, Wsk, Wsv, Wo):
    out, _ = _run(dict(
        hidden_states=hidden_states, ssm_states=ssm_states, Wq=Wq, Wk=Wk,
        Wv=Wv, Wsk=Wsk, Wsv=Wsv, Wo=Wo))
    return out


# revision 32
# speedup vs baseline: 1.0572x; 1.0084x over previous
"""DualSlidingWindowAttention Trainium2 kernel.

Sharding: 8 cores = 2 batches x 4 head-groups. Core (b, m) owns batch b,
q-heads 8m..8m+7, kv-heads 2m, 2m+1. Host sums the 4 partial o-proj outputs
per batch (f16 partials, f32 accumulate).

Single fused pipeline per core; the Tile scheduler overlaps phases because
tiles are split at the granularity readers consume them (per token-half)
and all pools coexist in SBUF/PSUM (no reuse barriers):
  - projections with weights stationary; kT/qT land score-ready, v is
    DMA-XBAR-transposed to [kv, D] (no PE/PSUM involved).
  - block-sparse attention per (kv-group, 128-query tile): 5 kv chunks,
    scores transposed [kv, q] with the group's 4 heads in the free dim.
    Softmax: exp(s/8 - 4) on ACT (bias keeps pre-norm o in f16 range),
    mask*exp(alibi) multiplied in f16 on DVE, softmax sums via a ones
    column appended to v (free on the PE), normalization per 2-qtile batch:
    reciprocal_approx_fast + DRAM-roundtrip broadcast + in-place f16 mul.
  - o-proj in 256-token slabs as each 2-qtile batch normalizes; f16 out.

DMA discipline: few, large, contiguous-row transfers; issue split across
the two HWDGE queues (sync: xt/weights/attention shuffles; scalar:
wq/wo/mconc/v-transposes) and GPSIMD SWDGE (output writes, memsets).
All matmul operands f16 (1 cycle/row), accumulation f32 in PSUM.
"""

import sys

sys.path.insert(0, "/opt/trn_rl_repo")

import numpy as np
import concourse.bass as bass
import concourse.bacc as bacc
import concourse.mybir as mybir
import concourse.tile as tile

F32 = mybir.dt.float32
F16 = mybir.dt.float16

HID, H, HK, G, D, T = 2048, 32, 8, 4, 64, 1024
W_ATT, W_SSM = 256, 64
NQT = T // 128  # 8 query tiles
KVG = 2         # kv heads (= head groups) per core
HL = 4          # q heads per kv group
EXP_BIAS = -4.0  # exp(s/8 + EXP_BIAS): keeps pre-norm o within f16 range

# slot order: [attn_left, ssm_left, attn_full, attn_causal, ssm_causal]
SLOT_SRC = [1, 0, 1, 1, 0]       # 1 = hidden (attn window), 0 = ssm
SLOT_CHOFF = [-2, -1, -1, 0, 0]  # kv chunk offset relative to qtile
SLOT_OFF = [-256, -128, -128, 0, 0]
SLOT_WIN = [W_ATT, W_SSM, W_ATT, W_ATT, W_SSM]


def first_slot(qt):
    return {0: 3, 1: 1}.get(qt, 0)


DEBUG_DUMPS = False


def build_program():
    nc = bacc.Bacc("TRN2", target_bir_lowering=False, debug=False)

    xt_ssm = nc.declare_dram_parameter("xt_ssm", [HID, T], F16, isOutput=False)
    xt_hid = nc.declare_dram_parameter("xt_hid", [HID, T], F16, isOutput=False)
    wq = nc.declare_dram_parameter("wq", [128, 32, 512], F16, isOutput=False)
    wk = nc.declare_dram_parameter("wk", [128, 16, 128], F16, isOutput=False)
    wv = nc.declare_dram_parameter("wv", [128, 16, 128], F16, isOutput=False)
    wsk = nc.declare_dram_parameter("wsk", [128, 16, 128], F16, isOutput=False)
    wsv = nc.declare_dram_parameter("wsv", [128, 16, 128], F16, isOutput=False)
    wo = nc.declare_dram_parameter("wo", [128, 4, 2048], F16, isOutput=False)
    mconc = nc.declare_dram_parameter("mconc", [128, 10, 512], F16, isOutput=False)
    out_t = nc.declare_dram_parameter("out_t", [HID, T], F16, isOutput=True)

    mm = nc.tensor.matmul
    xt_dram = [xt_ssm, xt_hid]

    if DEBUG_DUMPS:
        dbg_kT = nc.declare_dram_parameter("dbg_kT", [128, 4, 512], F16,
                                           isOutput=True)
        dbg_qT = nc.declare_dram_parameter("dbg_qT", [128, 8, 512], F16,
                                           isOutput=True)
        dbg_v = nc.declare_dram_parameter("dbg_v", [128, 32, 65], F16,
                                          isOutput=True)
        dbg_oT = nc.declare_dram_parameter("dbg_oT", [128, 4, T], F16,
                                           isOutput=True)
        dbg_sums = nc.declare_dram_parameter("dbg_sums", [16, 512], F32,
                                             isOutput=True)
        dbg_wei = nc.declare_dram_parameter("dbg_wei", [128, 80, 512], F16,
                                            isOutput=True)

    with tile.TileContext(nc) as tc:
        with (
            tc.tile_pool(name="persist", bufs=1) as pers,
            tc.tile_pool(name="dram", bufs=1, space="DRAM") as dramp,
            tc.tile_pool(name="stp", bufs=2) as stp,
            tc.tile_pool(name="weip", bufs=2) as weip,
            tc.tile_pool(name="ostgp", bufs=2) as ostgp,
            tc.tile_pool(name="sstgp", bufs=1) as sstgp,
            tc.tile_pool(name="outstgp", bufs=2) as outstgp,
            tc.tile_pool(name="rbcp", bufs=2) as rbcp,
            tc.tile_pool(name="recp", bufs=1) as recp,
            tc.tile_pool(name="otnp", bufs=2) as otnp,
            tc.tile_pool(name="kvqp", bufs=2, space="PSUM") as kvqp,
            tc.tile_pool(name="spp", bufs=2, space="PSUM") as spp,
            tc.tile_pool(name="opp", bufs=2, space="PSUM") as opp,
        ):
            # ---- persistent tiles (each tag its own slot) ----
            # xt[src][pc]: [128, 2 chunks, 1024 tokens]; chunk kc -> (kc//2, kc%2)
            xt_sb = [
                [pers.tile([128, 2, T], F16, tag=f"xt{s}_{pc}", name=f"xt{s}_{pc}")
                 for pc in range(8)]
                for s in range(2)
            ]
            wq_sb = pers.tile([128, 32, 512], F16, tag="wq")
            w4_names = ("wsk", "wsv", "wk", "wv")
            w4_t = {"wsk": wsk, "wsv": wsv, "wk": wk, "wv": wv}
            w4_sb = {n: pers.tile([128, 16, 128], F16, tag=n, name=n)
                     for n in w4_names}
            wo_sb = pers.tile([128, 4, 2048], F16, tag="wo")
            m_sb = pers.tile([128, 10, 512], F16, tag="mconc")
            # qT[half]: [128 (kvg,d), 4 qtiles, 512 (4 heads x 128 q)]
            qT_sb = [pers.tile([128, 4, 512], F16, tag=f"qT{h}", name=f"qT{h}")
                     for h in range(2)]
            # kT[src][half]: [128 (kvg,d), 512 tokens]
            kT_sb = [[pers.tile([128, 512], F16, tag=f"kT{s}{h}", name=f"kT{s}{h}")
                      for h in range(2)] for s in range(2)]
            # v[src][kvh][half]: [128 tok-in-chunk, 4 chunks, 128]; cols 0:64
            # = v, col 64 = ones (128-wide so DMA-transpose dst offsets stay
            # 256B-aligned; cols 65:128 unused)
            v_sb = [
                [[pers.tile([128, 4, 128], F16, tag=f"v{s}{kh}{h}",
                            name=f"v{s}{kh}{h}") for h in range(2)]
                 for kh in range(2)]
                for s in range(2)
            ]
            # oT: [128 (par,d), 4 (kvg,t), 1024 tokens] f16 pre-norm;
            # normalized 256-token slabs rotate through otnp
            oT_sb = pers.tile([128, 4, T], F16, tag="oT")
            otn_tiles = {}
            # per-pair sums tiles so reciprocal reads from partition base 0
            sums_sb = [pers.tile([4, 512], F32, tag=f"sums{p}", name=f"sums{p}")
                       for p in range(4)]
            rd = dramp.tile([16, 512], F16, tag="rd")
            ebias_sb = pers.tile([128, 1], F32, tag="ebias")
            nc.gpsimd.memset(ebias_sb[:, :], EXP_BIAS)

            # ones column of v (col 64)
            for s in range(2):
                for kh in range(2):
                    for h in range(2):
                        nc.gpsimd.memset(v_sb[s][kh][h][:, :, 64:65], 1.0)

            # ---- input DMAs ----
            # Completion time of any transfer ~ bytes outstanding ahead of it
            # (HW stripes service across queues), so order strictly by first
            # use: kv weights, then all ssm pieces (kv-ssm proj runs first),
            # then hid pieces. xt pieces are [128, 2, 1024] full-row blocks.
            for n in w4_names:
                nc.sync.dma_start(out=w4_sb[n], in_=w4_t[n][:, :, :])
            for s in range(2):
                for pc in range(8):
                    nc.sync.dma_start(
                        out=xt_sb[s][pc],
                        in_=xt_dram[s][pc * 256:(pc + 1) * 256, :].rearrange(
                            "(c p) t -> p c t", c=2))
            # big weight tables on the scalar HWDGE queue; wq split in 4 so
            # q-proj matmuls unlock as chunks land, wo (needed last) last
            for w in range(4):
                nc.scalar.dma_start(out=wq_sb[:, w * 8:(w + 1) * 8, :],
                                    in_=wq[:, w * 8:(w + 1) * 8, :])
            nc.scalar.dma_start(out=m_sb, in_=mconc[:, :, :])
            nc.scalar.dma_start(out=wo_sb, in_=wo[:, :, :])

            def xt_ap(src, kc, half):
                return xt_sb[src][kc // 2][:, kc % 2,
                                           half * 512:(half + 1) * 512]

            # ---- projections for one token half ----
            def emit_kv_proj(half, src):
                kps = kvqp.tile([128, 512], F32, tag="kv")
                vps = kvqp.tile([128, 512], F32, tag="kv")
                wk_t = w4_sb["wk" if src else "wsk"]
                wv_t = w4_sb["wv" if src else "wsv"]
                for kc in range(16):
                    xap = xt_ap(src, kc, half)
                    mm(kps[:, :], lhsT=wk_t[:, kc, :], rhs=xap,
                       start=(kc == 0), stop=(kc == 15))
                    mm(vps[:, :], lhsT=wv_t[:, kc, :], rhs=xap,
                       start=(kc == 0), stop=(kc == 15))
                nc.vector.tensor_copy(kT_sb[src][half][:, :], kps[:, :])
                stage = stp.tile([128, 512], F16, tag="stage")
                nc.vector.tensor_copy(stage[:, :], vps[:, :])
                # v: [d, tok] -> [tok, d] via DMA XBAR transpose; on sync,
                # where it queues behind the phase-A input stream and keeps
                # the scalar engine free for attention exps.
                for kh in range(2):
                    for j in range(4):
                        nc.sync.dma_start(
                            out=v_sb[src][kh][half][:, j, 0:64],
                            in_=stage[kh * 64:(kh + 1) * 64,
                                      j * 128:(j + 1) * 128],
                            transpose=True)

            def emit_q_proj(half, c):
                qps = kvqp.tile([128, 512], F32, tag="q")
                for src in range(2):
                    for kc in range(16):
                        mm(qps[:, :],
                           lhsT=wq_sb[:, src * 16 + kc, c * 128:(c + 1) * 128],
                           rhs=xt_ap(src, kc, half),
                           start=(src == 0 and kc == 0),
                           stop=(src == 1 and kc == 15))
                nc.vector.tensor_copy(
                    qT_sb[half][:, :, c * 128:(c + 1) * 128],
                    qps[:, :].rearrange("p (qt j) -> p qt j", j=128))

            # ---- attention ----
            units = [(qt * 2 + kvg, kvg, qt) for qt in range(NQT)
                     for kvg in range(KVG)]
            wei_tiles = {}

            def emit_scores(u):
                _, kvg, qt = units[u]
                fs = first_slot(qt)
                wei_t = weip.tile([128, 5, 512], F16, tag="wei")
                wei_tiles[u] = wei_t
                for s in range(fs, 5):
                    ch = qt + SLOT_CHOFF[s]
                    src = SLOT_SRC[s]
                    sp_t = spp.tile([128, 512], F32, tag="sp")
                    mm(sp_t[:, :],
                       lhsT=kT_sb[src][ch // 4][kvg * 64:(kvg + 1) * 64,
                                                (ch % 4) * 128:(ch % 4 + 1) * 128],
                       rhs=qT_sb[qt // 4][kvg * 64:(kvg + 1) * 64, qt % 4, :],
                       start=True, stop=True)
                    nc.scalar.activation(
                        out=wei_t[:, s, :], in_=sp_t[:, :],
                        func=mybir.ActivationFunctionType.Exp,
                        scale=0.125, bias=ebias_sb[:, :])
                nc.vector.tensor_mul(
                    wei_t[:, fs:5, :], wei_t[:, fs:5, :],
                    m_sb[:, kvg * 5 + fs:kvg * 5 + 5, :])
                if DEBUG_DUMPS:
                    nc.gpsimd.dma_start(
                        out=dbg_wei[:, u * 5 + fs:u * 5 + 5, :],
                        in_=wei_t[:, fs:5, :])

            def emit_o(u):
                _, kvg, qt = units[u]
                fs = first_slot(qt)
                wei_t = wei_tiles.pop(u)
                op_t = opp.tile([128, 512], F32, tag="op")
                for s in range(fs, 5):
                    ch = qt + SLOT_CHOFF[s]
                    src = SLOT_SRC[s]
                    mm(op_t[0:65, :],
                       lhsT=v_sb[src][kvg][ch // 4][:, ch % 4, 0:65],
                       rhs=wei_t[:, s, :],
                       start=(s == fs), stop=(s == 4))
                ostg = ostgp.tile([64, 512], F16, tag="ostg")
                nc.vector.tensor_copy(ostg[:, :], op_t[0:64, :])
                # sums row: psum f32 -> f32 staging at same partition base,
                # then DMA into this pair's sums tile
                sstg = sstgp.tile([65, 512], F32, tag="sstg")
                nc.vector.tensor_copy(sstg[64:65, :], op_t[64:65, :])
                nc.sync.dma_start(out=sums_sb[u // 4][u % 4:u % 4 + 1, :],
                                  in_=sstg[64:65, :])
                for par in range(2):
                    src_ap = ostg[:, :].rearrange(
                        "p (t pr j) -> p t pr j", t=2, pr=2)[:, :, par, :]
                    dst_ap = oT_sb[par * 64:(par + 1) * 64,
                                   kvg * 2:kvg * 2 + 2,
                                   qt * 128:(qt + 1) * 128]
                    nc.sync.dma_start(out=dst_ap, in_=src_ap)

            def emit_norm(p):
                # qtiles 2p, 2p+1 = units 4p..4p+3 (sums rows likewise)
                r32 = recp.tile([4, 512], F32, tag="rf32")
                r16 = recp.tile([4, 512], F16, tag="rf16")
                nc.vector.reciprocal_approx_fast(
                    out=r32[:, :], in_=sums_sb[p][:, :])
                nc.vector.tensor_copy(r16[:, :], r32[:, :])
                nc.sync.dma_start(out=rd[4 * p:4 * p + 4, :], in_=r16[:, :])
                rbc_t = rbcp.tile([128, 4, 256], F16, tag="rbc")
                rd_ap = rd[:, :]
                for par in range(2):
                    for qtloc in range(2):
                        # value for (d, c=(kvg,t), j) at rd row
                        # 4p + 2*qtloc + kvg, col t*256 + par*128 + j;
                        # c strides uniformly by 256 (kvg row-stride 512 =
                        # 2x the t col-stride 256), so 3 dims suffice.
                        in_ap = bass.AP(
                            tensor=rd_ap.tensor,
                            offset=(rd_ap.offset + (4 * p + 2 * qtloc) * 512
                                    + par * 128),
                            ap=[[0, 64], [256, 4], [1, 128]])
                        nc.sync.dma_start(
                            out=rbc_t[par * 64:(par + 1) * 64, :,
                                      qtloc * 128:(qtloc + 1) * 128],
                            in_=in_ap)
                otn = otnp.tile([128, 4, 256], F16, tag="otn")
                otn_tiles[p] = otn
                nc.vector.tensor_mul(
                    otn[:, :, :],
                    oT_sb[:, :, p * 256:(p + 1) * 256],
                    rbc_t[:, :, :])

            def emit_oproj_g(p, g):
                # tokens [256p, 256p+256), output cols [512g, 512g+512)
                otn = otn_tiles[p]
                outstg = outstgp.tile([128, 4, 256], F16, tag="outstg")
                for i in range(4):
                    n = g * 4 + i
                    p3 = kvqp.tile([128, 256], F32, tag="kv")
                    for c in range(4):
                        mm(p3[:, :],
                           lhsT=wo_sb[:, c, n * 128:(n + 1) * 128],
                           rhs=otn[:, c, :],
                           start=(c == 0), stop=(c == 3))
                    if i % 2 == 0:
                        nc.scalar.copy(outstg[:, i, :], p3[:, :])
                    else:
                        nc.vector.tensor_copy(outstg[:, i, :], p3[:, :])
                dst = out_t[g * 512:(g + 1) * 512,
                            p * 256:(p + 1) * 256].rearrange(
                                "(i p2) t -> p2 i t", i=4)
                nc.gpsimd.dma_start(out=dst, in_=outstg[:, :, :])

            def emit_oproj(p):
                for g in range(4):
                    emit_oproj_g(p, g)
                del otn_tiles[p]

            # ---------------- emission schedule ----------------
            # half 0 projections
            for src in range(2):
                emit_kv_proj(0, src)
            for c in range(4):
                emit_q_proj(0, c)

            # half-1 projections interleaved with attention on qtiles 0-3;
            # the scheduler refines this, the order sets priorities.
            emit_kv_proj(1, 0)
            emit_scores(0)
            emit_kv_proj(1, 1)
            emit_scores(1)
            emit_o(0)
            emit_q_proj(1, 0)
            emit_scores(2)
            emit_o(1)
            emit_q_proj(1, 1)
            emit_scores(3)
            emit_o(2)
            emit_q_proj(1, 2)
            emit_scores(4)
            emit_o(3)
            emit_norm(0)
            emit_q_proj(1, 3)
            emit_scores(5)
            emit_o(4)
            emit_scores(6)
            emit_o(5)
            emit_scores(7)
            emit_o(6)

            # qtiles 4-7 + norms + o-proj slabs pipelined; o-proj emitted in
            # 16-matmul groups between units to keep every engine fed
            emit_scores(8)
            emit_o(7)
            emit_norm(1)
            emit_oproj_g(0, 0)
            emit_oproj_g(0, 1)
            sched = {9: [(0, 2), (0, 3)], 10: [(1, 0), (1, 1)],
                     11: [(1, 2)], 12: [(1, 3)],
                     13: [(2, 0)], 14: [(2, 1), (2, 2)], 15: [(2, 3)]}
            for u in range(9, 16):
                emit_scores(u)
                emit_o(u - 1)
                if u == 13:
                    emit_norm(2)
                for (pp, g) in sched.get(u, []):
                    emit_oproj_g(pp, g)
            emit_o(15)
            emit_norm(3)
            emit_oproj(3)

            if DEBUG_DUMPS:
                for s in range(2):
                    for h in range(2):
                        nc.gpsimd.dma_start(
                            out=dbg_kT[:, s * 2 + h, :], in_=kT_sb[s][h][:, :])
                for h in range(2):
                    nc.gpsimd.dma_start(
                        out=dbg_qT[:, h * 4:(h + 1) * 4, :],
                        in_=qT_sb[h][:, :, :])
                for s in range(2):
                    for kh in range(2):
                        for h in range(2):
                            idx = (s * 2 + kh) * 2 + h
                            nc.gpsimd.dma_start(
                                out=dbg_v[:, idx * 4:(idx + 1) * 4, :],
                                in_=v_sb[s][kh][h][:, :, 0:65])
                nc.gpsimd.dma_start(out=dbg_oT[:, :, :], in_=oT_sb[:, :, :])
                for p in range(4):
                    nc.gpsimd.dma_start(out=dbg_sums[4 * p:4 * p + 4, :],
                                        in_=sums_sb[p][:, :])

    nc.finalize()
    return nc


def make_mconc(m):
    """Mask*exp(alibi) tile for core head-group m: [128, 10, 512] f16."""
    p = np.arange(128)[:, None]
    j = np.arange(128)[None, :]
    out = np.zeros((128, 10, 512), np.float16)
    for kvg in range(KVG):
        for s in range(5):
            rel = SLOT_OFF[s] + p - j  # [128, 128] kv - q
            mask = (-rel >= 0) & (-rel < SLOT_WIN[s])
            for hl in range(HL):
                hg = 8 * m + kvg * 4 + hl
                slope = 2.0 ** (-8.0 * hg / H)
                vals = np.where(mask, np.exp(slope * rel.astype(np.float64)), 0.0)
                out[:, kvg * 5 + s, hl * 128:(hl + 1) * 128] = vals.astype(np.float16)
    return out


def make_inputs(core, hidden_states, ssm_states, Wq, Wk, Wv, Wsk, Wsv, Wo):
    b, m = core // 4, core % 4
    f16 = lambda x: np.ascontiguousarray(np.asarray(x, dtype=np.float16))

    def wshard(W, cols, nchunk):
        # [K, cols] -> [128, K//128, cols]
        Ws = np.asarray(W)[:, cols]
        return f16(Ws.reshape(nchunk, 128, Ws.shape[1]).transpose(1, 0, 2))

    # col-tile c = [head c (kvg0) cols, head 4+c (kvg1) cols]
    qperm = np.concatenate(
        [np.arange(64) + 64 * h for c in range(4) for h in (c, 4 + c)])
    qcols = 512 * m + qperm
    kvcols = slice(128 * m, 128 * (m + 1))
    wo_sh = np.asarray(Wo)[512 * m:512 * (m + 1), :]
    return {
        "xt_ssm": f16(np.asarray(ssm_states)[b].T),
        "xt_hid": f16(np.asarray(hidden_states)[b].T),
        "wq": wshard(Wq, qcols, 32),
        "wk": wshard(Wk, kvcols, 16),
        "wv": wshard(Wv, kvcols, 16),
        "wsk": wshard(Wsk, kvcols, 16),
        "wsv": wshard(Wsv, kvcols, 16),
        "wo": f16(wo_sh.reshape(4, 128, 2048).transpose(1, 0, 2)),
        "mconc": make_mconc(m),
    }


def gather(results):
    out = np.zeros((2, T, HID), np.float32)
    for core in range(8):
        b = core // 4
        out[b] += results[core]["out_t"].astype(np.float32).T
    return out


# ----------------------------------------------------------------------------
# Harness entry point
# ----------------------------------------------------------------------------
_NC_CACHE = []


def _get_program():
    if not _NC_CACHE:
        _NC_CACHE.append(build_program())
    return _NC_CACHE[0]


def _run(inp, trace=False):
    from concourse.bass_utils import run_bass_kernel_spmd

    nc = _get_program()
    in_maps = [make_inputs(core, **{k: np.asarray(inp[k]) for k in (
        "hidden_states", "ssm_states", "Wq", "Wk", "Wv", "Wsk", "Wsv", "Wo")})
        for core in range(8)]
    res = run_bass_kernel_spmd(nc, in_maps, list(range(8)), trace=trace)
    return gather(res.results), res.exec_time_ns


def kernel(hidden_states, ssm_states, Wq, Wk, Wv, Wsk, Wsv, Wo):
    out, _ = _run(dict(
        hidden_states=hidden_states, ssm_states=ssm_states, Wq=Wq, Wk=Wk,
        Wv=Wv, Wsk=Wsk, Wsv=Wsv, Wo=Wo))
    return out
